# revision 11
# baseline (speedup 1.0000x reference)
"""CorrNoise kernel for 8x TRN2 NeuronCores.

Reference computation: center/normalize ref over batch -> per-dim (l x l)
correlation -> eigh -> out[d] = (Q*sqrt(max(eig,0)))[d] @ noise[d].

Split of work:
  * corr + eigh run on HOST with jax on CPU, mirroring the reference ops
    bit-exactly.  This is forced: (a) eigh has no neuron lowering at all;
    (b) LAPACK eigenvector SIGNS are implementation-defined and flip under
    ~1e-7 input perturbations, and the output is sign-sensitive, so the
    eigh input must be bit-identical to the reference's and the eigh must
    be the same LAPACK build (jnp.linalg.eigh on CPU).
  * The post-eigh work - 512 independent (128x128)@(128x256) GEMMs - runs
    on the 8 NeuronCores, sharded by dim (64 per core).

Device kernel design (measured on HW via NTFF profiles):
  * The kernel is DMA-byte-bound: the DMA fabric saturates at ~400-425
    GB/s aggregate (16 engines x ~25 GB/s, flat in packet size >= 2KB)
    for the whole steady state; tensor/vector/scalar stay under.  The
    rel-err budget is 2e-2, so precision is spent where it buys bytes:
      - noise ships as INT8 (1 B/elem) with a per-dim symmetric scale
        s_d = max|noise_d|/127.  The scale is folded into that dim's QS
        matrix on the host (QS_d * s_d, fp16), so the device only does a
        pure int8->fp16 tensor_copy expansion and the fp16 matmul sees
        exact integer noise values (<=127, exact in fp16).
      - QS ships fp16, output ships fp16 (cast in the PSUM->SBUF copy).
    Measured end-to-end rel err 0.98e-2 vs the 2e-2 budget; all error
    terms are deterministic quantization, not run-to-run noise.
    Traffic: 8.4 MB/core (2.1 qs + 2.1 noise + 4.2 out) vs 21 MB for the
    fp32-accurate variant and 10.5 MB for all-fp16.
  * Flat [128, N] DRAM layouts, chunked so every DMA has 4-8 KB
    contiguous rows; inputs interleave qs/noise per 16-dim group so
    expansion+matmul pipeline behind the input stream.  All payload DMAs
    issue on the sync ring (outputs enqueue behind inputs, each gated on
    its group's copies); the last store is split with scalar to shorten
    the drain.
  * Engine budget per 16-dim group (arrival period ~2.6 us): int8
    expansion on vector; 4 PSUM copy-out tiles (4 dims each) split
    scalar/gpsimd/scalar/vector.  PSUM pool: 4 tiles x 2 banks = all 8
    banks.
"""

import numpy as np

EPS = 1e-5
SIZE = 128   # l: corr matrices are SIZE x SIZE
DIM = 512    # d: number of independent feature dims
BATCH = 256  # b
NCORES = 8
DPC = DIM // NCORES  # dims per core
GD = 16              # dims per group (input-chunk + output-chunk unit)
NG = DPC // GD       # 4 groups
PSD = 4              # dims per PSUM tile / per copy-out

_cache = {}


def _host_qs(ref: np.ndarray) -> np.ndarray:
    """Bit-exact mirror of the reference's pre-matmul stages on jax CPU.

    Returns QS = Ds[:, None, :] * Qs with shape (DIM, SIZE, SIZE), fp32.
    """
    import jax
    import jax.numpy as jnp

    cpu = jax.devices("cpu")[0]
    with jax.default_device(cpu):
        refj = jnp.asarray(np.asarray(ref, dtype=np.float32))
        x = refj - refj.mean(axis=0, keepdims=True)
        x = x / (jnp.linalg.norm(x, axis=0, keepdims=True) + EPS)
        x = jnp.transpose(x, (2, 1, 0))  # (d, l, b)
        corr = jnp.einsum("dlb,dmb->dlm", x, x)  # (d, l, l)
        i = jnp.arange(SIZE)
        corr = corr.at[:, i, i].set(1.0)
        Ds, Qs = jnp.linalg.eigh(corr)  # Ds: (d, l), Qs: (d, l, l)
        Ds = jnp.sqrt(jnp.maximum(Ds, 0.0))
        Qs = Ds[:, None, :] * Qs
        return np.asarray(Qs)


def _build_nc():
    import concourse.bass as bass
    import concourse.tile as tile
    from concourse import bacc, mybir

    f32 = mybir.dt.float32
    f16 = mybir.dt.float16
    i8 = mybir.dt.int8
    QW = DPC * SIZE    # qs cols (f16): 16 KB rows, 4 KB per group
    NW = DPC * BATCH   # noise cols (i8): 16 KB rows, 4 KB per group
    OW = DPC * BATCH   # out cols (f16): 32 KB rows, 8 KB per group
    QG = GD * SIZE     # qs cols per group
    NGC = GD * BATCH   # noise cols per group
    nc = bacc.Bacc("TRN2", target_bir_lowering=False, debug=False,
                   num_devices=NCORES)
    qs = nc.dram_tensor("qs", [SIZE, QW], f16, kind="ExternalInput").ap()
    nz = nc.dram_tensor("nz", [SIZE, NW], i8, kind="ExternalInput").ap()
    out = nc.dram_tensor("out", [SIZE, OW], f16, kind="ExternalOutput").ap()
    EV = 8 * BATCH   # expansion cols per group on vector (0.59 ns/col)
    ES = 6 * BATCH   # on scalar (1.0 ns/col); gpsimd takes the last 2 dims
    with tile.TileContext(nc) as tc:
        with (
            tc.tile_pool(name="q", bufs=1) as qp,
            tc.tile_pool(name="n8", bufs=1) as n8p,
            tc.tile_pool(name="nf", bufs=1) as nfp,
            tc.tile_pool(name="o", bufs=NG) as op_,
            tc.tile_pool(name="ps", bufs=4, space=bass.MemorySpace.PSUM) as pp,
        ):
            tq = qp.tile([SIZE, QW], f16)
            t8 = n8p.tile([SIZE, NW], i8)
            tf = nfp.tile([SIZE, NW], f16)
            for g in range(NG):
                nc.sync.dma_start(t8[:, g * NGC:(g + 1) * NGC],
                                  nz[:, g * NGC:(g + 1) * NGC])
                nc.sync.dma_start(tq[:, g * QG:(g + 1) * QG],
                                  qs[:, g * QG:(g + 1) * QG])
            for g in range(NG):
                # int8 -> fp16 expansion (values are exact integers <= 127;
                # the quantization scale lives in qs already).  Split so
                # vector (0.59 ns/col) / scalar (1.0) / gpsimd (4.0) finish
                # together given the copy-out shares below.
                b = g * NGC
                nc.vector.tensor_copy(tf[:, b:b + EV], t8[:, b:b + EV])
                nc.scalar.copy(tf[:, b + EV:b + EV + ES],
                               t8[:, b + EV:b + EV + ES])
                nc.gpsimd.tensor_copy(tf[:, b + EV + ES:b + NGC],
                                      t8[:, b + EV + ES:b + NGC])
                o = op_.tile([SIZE, NGC], f16)
                for pj in range(GD // PSD):
                    ps = pp.tile([SIZE, PSD * BATCH], f32)
                    for k in range(PSD):
                        j = g * GD + pj * PSD + k
                        w = tq[:, j * SIZE:(j + 1) * SIZE]
                        x = tf[:, j * BATCH:(j + 1) * BATCH]
                        nc.tensor.matmul(ps[:, k * BATCH:(k + 1) * BATCH],
                                         w, x, start=True, stop=True)
                    dst = o[:, pj * PSD * BATCH:(pj + 1) * PSD * BATCH]
                    if pj % 2 == 0:
                        nc.scalar.copy(dst, ps[:])
                    else:
                        nc.vector.tensor_copy(dst, ps[:])
                    if g == NG - 1:
                        # last group: store per copy-out tile so the final
                        # drain overlaps the remaining copy-outs
                        lo = g * NGC + pj * PSD * BATCH
                        nc.sync.dma_start(out[:, lo:lo + PSD * BATCH], dst)
                if g < NG - 1:
                    nc.sync.dma_start(out[:, g * NGC:(g + 1) * NGC], o[:])
    nc.compile()
    return nc


def _run_device(qst: np.ndarray, noise_t: np.ndarray, trace: bool = False):
    """qst: (DIM, SIZE, SIZE) = QS transposed per dim (fp32);
    noise_t: (DIM, SIZE, BATCH) fp32.
    Returns (out_t (DIM, SIZE, BATCH) fp32, BassKernelResults)."""
    from concourse.bass_utils import run_bass_kernel_spmd

    if "nc" not in _cache:
        _cache["nc"] = _build_nc()
    nc = _cache["nc"]

    # per-dim symmetric int8 quantization of noise; scale folded into qs
    s = np.abs(noise_t).max(axis=(1, 2), keepdims=True) / 127.0  # (DIM,1,1)
    nz8 = np.clip(np.round(noise_t / s), -127, 127).astype(np.int8)
    qsw = (qst * s).astype(np.float16)  # (DIM, SIZE, SIZE)

    qs_l = qsw.reshape(NCORES, DPC, SIZE, SIZE).transpose(0, 2, 1, 3)
    qs_l = np.ascontiguousarray(qs_l).reshape(NCORES, SIZE, DPC * SIZE)
    nz_l = nz8.reshape(NCORES, DPC, SIZE, BATCH).transpose(0, 2, 1, 3)
    nz_l = np.ascontiguousarray(nz_l).reshape(NCORES, SIZE, DPC * BATCH)
    in_maps = [{"qs": qs_l[c], "nz": nz_l[c]} for c in range(NCORES)]
    res = run_bass_kernel_spmd(nc, in_maps, list(range(NCORES)), trace=trace)
    out_t = np.stack([res.results[c]["out"] for c in range(NCORES)])
    out_t = out_t.reshape(NCORES, SIZE, DPC, BATCH)
    out_t = out_t.transpose(0, 2, 1, 3).reshape(DIM, SIZE, BATCH)
    return out_t.astype(np.float32), res


def kernel(standard_noise: np.ndarray, ref: np.ndarray) -> np.ndarray:
    qs = _host_qs(ref)  # (d, l, l)
    qst = np.ascontiguousarray(np.transpose(qs, (0, 2, 1)))
    noise_t = np.ascontiguousarray(
        np.transpose(np.asarray(standard_noise, dtype=np.float32), (2, 1, 0)))
    out_t, _ = _run_device(qst, noise_t)
    return np.ascontiguousarray(np.transpose(out_t, (2, 1, 0)))
